# revision 18
# baseline (speedup 1.0000x reference)
"""Mistral decoder layer (B=1, S=1024, HID=4096, 32 heads, INTER=11008),
tensor-parallel over 8 trn2 NeuronCores (Megatron style).

v2 redesign (from NTFF trace analysis of the fp32 baseline):
  - PE clock is throttled to ~1.2-1.7GHz effective regardless of dtype, so
    matmul time is fixed; the wins are overlap + memory:
  - f16 storage/operands everywhere (psum accumulation stays fp32):
    halves weight DMA (101MB -> 51MB/core), keeps x resident (no 16MB
    reload at o-proj), h2 consumed directly from the f16 AllReduce (no
    upcast), m held in SBUF (no DRAM bounce). et/V use bf16 (exp range).
  - host pre-lays-out every weight so each DMA is >=1KB-contiguous rows.
  - attention(c) interleaved with o-proj(c); o-proj AllReduce chunked per
    column-half (2 x 4MB) so AR(c0) hides under attn(c1)+o(c1) and AR(c1)
    under up/gate(c0); down AllReduce chunked 4 x 2MB to shrink the tail.
  - RMSNorm stats via ones-matmuls (fp32r), gains + 1/sqrt(hd) folded into
    weights on host; residuals folded into the AllReduces (x/8, h2/8).
"""

import numpy as np

import concourse.bacc as bacc
import concourse.mybir as mybir
import concourse.tile as tile
from concourse.bass_utils import run_bass_kernel_spmd

AF = mybir.ActivationFunctionType
ALU = mybir.AluOpType
F32 = mybir.dt.float32
F32R = mybir.dt.float32r
F16 = mybir.dt.float16
BF16 = mybir.dt.bfloat16

N_CORES = 8
HID = 4096
S = 1024
NH = 32
HD = 128
NH_L = NH // N_CORES          # 4 local heads
DL = NH_L * HD                # 512 local q/k/v dims
INTER = 11008
IL_T = 11                     # local intermediate k-tiles (padded)
IL = IL_T * 128               # 1408 padded local intermediate
ILR = INTER // N_CORES        # 1376 real local intermediate
KT = HID // 128               # 32 hidden k-tiles
CH = 2                        # seq chunks
CW = S // CH                  # 512
TB = S // 128                 # 8 seq tiles of 128
EPS = 1e-5

_CACHE = {}


def _r(ap):
    return ap.bitcast(F32R)


def _build(collectives=True, repeat=1):
    nc = bacc.Bacc("TRN2", target_bir_lowering=False, debug=False,
                   num_devices=N_CORES)

    xT = nc.dram_tensor("xT", [HID, S], F16, kind="ExternalInput").ap()
    maskTd = nc.dram_tensor("maskTd", [TB, 128, CW], BF16,
                            kind="ExternalInput").ap()
    wqT = nc.dram_tensor("wqT", [HID, DL], F16, kind="ExternalInput").ap()
    wkT = nc.dram_tensor("wkT", [HID, DL], F16, kind="ExternalInput").ap()
    wvT = nc.dram_tensor("wvT", [HID, DL], F16, kind="ExternalInput").ap()
    woT = nc.dram_tensor("woT", [DL, HID], F16, kind="ExternalInput").ap()
    wug = nc.dram_tensor("wug", [IL_T, 2, 128, HID], F16,
                         kind="ExternalInput").ap()
    wdn = nc.dram_tensor("wdn", [KT, 128, IL], F16, kind="ExternalInput").ap()
    outT = nc.dram_tensor("outT", [HID, S], F16, kind="ExternalOutput").ap()

    # o-proj AR: one chunk per column-half c: [HID, CW] f16 (4MB)
    ob = [nc.dram_tensor(f"ob{c}", [HID, CW], F16).ap() for c in range(CH)]
    h2d = [nc.dram_tensor(f"h2d{c}", [HID, CW], F16, addr_space="Shared").ap()
           for c in range(CH)]
    s1_d = nc.dram_tensor("s1_d", [S], F32).ap()
    # down AR chunks: (row-range, col-chunk) pairs; trailing chunks are
    # smaller (1MB) to shrink the tail-AR exposure.
    DN_CHUNKS = [  # (c, row0, row1)
        (0, 0, 2048), (0, 2048, 4096), (1, 0, 2048),
        (1, 2048, 3072), (1, 3072, 4096),
    ]
    dnb = [nc.dram_tensor(f"dnb{j}", [r1 - r0, CW], F16).ap()
           for j, (c, r0, r1) in enumerate(DN_CHUNKS)]
    dnr = [nc.dram_tensor(f"dnr{j}", [r1 - r0, CW], F16,
                          addr_space="Shared").ap()
           for j, (c, r0, r1) in enumerate(DN_CHUNKS)]

    rg = [list(range(N_CORES))]

    def all_reduce(dst, srcs):
        if collectives:
            nc.gpsimd.collective_compute(
                "AllReduce", ALU.add, ins=[srcs[:]], outs=[dst[:]],
                replica_groups=rg)
        else:
            nc.gpsimd.dma_start(dst[:], srcs[:])

    with tile.TileContext(nc) as tc:
      for rep in range(repeat):
        P = f"r{rep}_" if repeat > 1 else ""
        with tc.tile_pool(name=P + "const", bufs=1) as const:
            onesb = const.tile([128, 128], BF16, tag="onesb")
            nc.vector.memset(onesb[:], 1.0)
            s1 = const.tile([128, S], F32, tag="s1")
            s1t = const.tile([128, TB], F32, tag="s1t")
            s2 = const.tile([128, S], F32, tag="s2")
            epst = const.tile([128, 1], F32, tag="epst")
            nc.vector.memset(epst[:], EPS)

            with tc.tile_pool(name=P + "qkvo", bufs=1) as qkvo:
                QTt = [qkvo.tile([128, S], F16, tag=f"QT{h}", name=f"QT{h}")
                       for h in range(NH_L)]
                KTt = [qkvo.tile([128, S], F16, tag=f"KT{h}", name=f"KT{h}")
                       for h in range(NH_L)]
                Vt = [qkvo.tile([128, DL], BF16, tag=f"V{t}", name=f"V{t}")
                      for t in range(TB)]
                ATt = [qkvo.tile([128, S], F16, tag=f"AT{h}", name=f"AT{h}")
                       for h in range(NH_L)]
                wo_r = [qkvo.tile([128, HID], F16, tag=f"wo{h}", name=f"wo{h}")
                        for h in range(NH_L)]
                mtiles = [qkvo.tile([128, CW], BF16, tag=f"m{t}", name=f"mk{t}")
                          for t in range(TB)]

                with tc.tile_pool(name=P + "xres", bufs=1) as xres:
                    xt = [xres.tile([128, S], F16, tag=f"x{k}", name=f"x{k}")
                          for k in range(KT)]
                    # ---- phase 0: x load + RMSNorm#1 stats ----
                    with (
                        tc.tile_pool(name=P + "p0", bufs=2) as p0,
                        tc.tile_pool(name=P + "p0m", bufs=2) as p0m,
                        tc.tile_pool(name=P + "p0ps", bufs=1, space="PSUM") as p0ps,
                    ):
                        r2 = [p0ps.tile([128, CW], F32, tag=f"r2_{c}",
                                        name=f"r2_{c}") for c in range(CH)]
                        for k in range(KT):
                            eng = nc.sync if k % 2 == 0 else nc.gpsimd
                            eng.dma_start(xt[k][:],
                                          xT[k * 128:(k + 1) * 128, :])
                            sq = p0.tile([128, S], BF16, tag="sq", name=f"sq{k}")
                            nc.vector.tensor_mul(sq[:], xt[k][:], xt[k][:])
                            for c in range(CH):
                                nc.tensor.matmul(
                                    r2[c][:], onesb[:],
                                    sq[:, c * CW:(c + 1) * CW],
                                    start=(k == 0), stop=(k == KT - 1))
                        for c in range(CH):
                            ms = p0m.tile([128, CW], F32, tag="ms")
                            nc.scalar.activation(ms[:], r2[c][:], AF.Sqrt,
                                                 bias=epst[:], scale=1.0 / HID)
                            nc.vector.reciprocal(s1[:, c * CW:(c + 1) * CW],
                                                 ms[:])
                    # mask + resident wo load on the gpsimd queue (ahead of
                    # the s1 bounce, which waits on the stats)
                    for t in range(TB):
                        nc.gpsimd.dma_start(mtiles[t][:], maskTd[t, :, :])
                    for h in range(NH_L):
                        nc.gpsimd.dma_start(wo_r[h][:],
                                            woT[h * 128:(h + 1) * 128, :])
                    # s1t = s1 transposed down partitions, via a DRAM bounce
                    nc.gpsimd.dma_start(s1_d.rearrange("(o s) -> o s", o=1),
                                        s1[0:1, :])
                    nc.gpsimd.dma_start(s1t[:],
                                        s1_d.rearrange("(t p) -> p t", p=128))

                    # ---- phase 1: q/k passes (weights stream) ----
                    for nm, wT, outs in (("q", wqT, QTt), ("k", wkT, KTt)):
                        with (
                            tc.tile_pool(name=P + f"{nm}w", bufs=3) as wp,
                            tc.tile_pool(name=P + f"{nm}ps", bufs=1,
                                         space="PSUM") as ps,
                        ):
                            pt = [ps.tile([128, CW], F32, tag=f"pt{j}",
                                          name=f"pt{j}") for j in range(NH_L * CH)]
                            for k in range(KT):
                                wt = wp.tile([128, DL], F16, tag="wt")
                                nc.sync.dma_start(
                                    wt[:], wT[k * 128:(k + 1) * 128, :])
                                for h in range(NH_L):
                                    for c in range(CH):
                                        nc.tensor.matmul(
                                            pt[h * CH + c][:],
                                            wt[:, h * 128:(h + 1) * 128],
                                            xt[k][:, c * CW:(c + 1) * CW],
                                            start=(k == 0), stop=(k == KT - 1))
                            for h in range(NH_L):
                                for c in range(CH):
                                    nc.vector.tensor_mul(
                                        outs[h][:, c * CW:(c + 1) * CW],
                                        pt[h * CH + c][:],
                                        s1[:, c * CW:(c + 1) * CW])

                    # v pass: V[t] rows scaled by s1t column
                    with (
                        tc.tile_pool(name=P + "vw", bufs=3) as wp,
                        tc.tile_pool(name=P + "vps", bufs=1, space="PSUM") as ps,
                    ):
                        pt = [ps.tile([128, DL], F32, tag=f"pt{t}", name=f"pt{t}")
                              for t in range(TB)]
                        for k in range(KT):
                            wt = wp.tile([128, DL], F16, tag="wt")
                            nc.sync.dma_start(
                                wt[:], wvT[k * 128:(k + 1) * 128, :])
                            for t in range(TB):
                                nc.tensor.matmul(
                                    pt[t][:], xt[k][:, t * 128:(t + 1) * 128],
                                    wt[:], start=(k == 0), stop=(k == KT - 1))
                        for t in range(TB):
                            nc.vector.tensor_scalar(
                                Vt[t][:], pt[t][:], s1t[:, t:t + 1], None,
                                op0=ALU.mult)

                    # ---- phases 2+3 interleaved per column-half c:
                    #      attention(c) then o-proj(c) -> AR chunk c ----
                    with (
                        tc.tile_pool(name=P + "est", bufs=3) as estp,
                        tc.tile_pool(name=P + "etm", bufs=1) as etmp,
                        tc.tile_pool(name=P + "rin", bufs=2) as rinp,
                        tc.tile_pool(name=P + "aps", bufs=1, space="PSUM") as aps,
                        tc.tile_pool(name=P + "stps", bufs=3, space="PSUM") as stps,
                        tc.tile_pool(name=P + "ops", bufs=2, space="PSUM") as ops,
                        tc.tile_pool(name=P + "oev", bufs=4) as oev,
                    ):
                        atp = [aps.tile([128, CW], F32, tag=f"atp{j}",
                                        name=f"atp{j}") for j in range(2)]
                        rsp = aps.tile([128, CW], F32, tag="rsp", name="rsp")
                        # mask -> exp(mask) in place: masked et becomes a
                        # cheap bf16 multiply, and exp always reads PSUM
                        # directly (no DVE add on the exp critical path).
                        for t in range(TB):
                            nc.scalar.activation(mtiles[t][:], mtiles[t][:],
                                                 AF.Exp)

                        def emit_st(c, h, ets):
                            cs = slice(c * CW, (c + 1) * CW)
                            for t in range(0, (c + 1) * 4):
                                stp = stps.tile([128, CW], F32, tag="st")
                                nc.tensor.matmul(
                                    stp[:], KTt[h][:, t * 128:(t + 1) * 128],
                                    QTt[h][:, cs], start=True, stop=True)
                                et = etmp.tile([128, CW], BF16,
                                               tag=f"et{h}_{t}",
                                               name=f"et{h}_{t}")
                                if t >= c * 4:
                                    es = estp.tile([128, CW], BF16, tag="es")
                                    nc.scalar.activation(es[:], stp[:], AF.Exp)
                                    nc.vector.tensor_mul(et[:], es[:],
                                                         mtiles[t][:])
                                else:
                                    nc.scalar.activation(et[:], stp[:], AF.Exp)
                                ets[(h, t)] = et

                        def emit_pv(c, h, ets):
                            cs = slice(c * CW, (c + 1) * CW)
                            tbs = list(range(0, (c + 1) * 4))
                            ap_ = atp[h % 2]
                            for j, t in enumerate(tbs):
                                st_, sp_ = (j == 0), (j == len(tbs) - 1)
                                nc.tensor.matmul(
                                    ap_[:], Vt[t][:, h * 128:(h + 1) * 128],
                                    ets[(h, t)][:], start=st_, stop=sp_)
                                nc.tensor.matmul(
                                    rsp[:], onesb[:], ets[(h, t)][:],
                                    start=st_, stop=sp_)
                            ri = rinp.tile([128, CW], F32, tag="ri")
                            nc.vector.reciprocal(ri[:], rsp[:])
                            nc.vector.tensor_mul(ATt[h][:, cs], ap_[:], ri[:])

                        for c in range(CH):
                            cs = slice(c * CW, (c + 1) * CW)
                            # software-pipelined: STs run ahead of PV chains
                            # so exp/mul latency hides under tensor work.
                            ets = {}
                            emit_st(c, 0, ets)
                            emit_st(c, 1, ets)
                            emit_st(c, 2, ets)
                            emit_pv(c, 0, ets)
                            emit_st(c, 3, ets)
                            emit_pv(c, 1, ets)
                            emit_pv(c, 2, ets)
                            emit_pv(c, 3, ets)
                            # o-proj for this column-half + x/8 fold
                            for mh in range(KT):
                                pt = ops.tile([128, CW], F32, tag="pt")
                                for h in range(NH_L):
                                    nc.tensor.matmul(
                                        pt[:],
                                        wo_r[h][:, mh * 128:(mh + 1) * 128],
                                        ATt[h][:, cs],
                                        start=(h == 0), stop=(h == NH_L - 1))
                                ev = oev.tile([128, CW], F16, tag="ev")
                                nc.vector.scalar_tensor_tensor(
                                    ev[:], xt[mh][:, cs], 1.0 / N_CORES, pt[:],
                                    op0=ALU.mult, op1=ALU.add)
                                nc.gpsimd.dma_start(
                                    ob[c][mh * 128:(mh + 1) * 128, :], ev[:])
                            all_reduce(h2d[c], ob[c])

            # ---- phases 4+5: RMSNorm#2 stats + up/gate per column-half,
            #      then down-proj + chunked AR ----
            with tc.tile_pool(name=P + "h2res", bufs=1) as h2p:
                # h2 held as quad tiles [128, 4*CW] per (kq, c): one DMA per
                # quad (4x fewer dispatches on the AR->MLP critical path)
                h2q = {}
                for kq in range(KT // 4):
                    for c in range(CH):
                        h2q[(kq, c)] = h2p.tile([128, 4 * CW], F16,
                                                tag=f"h2q{kq}_{c}",
                                                name=f"h2q{kq}_{c}")

                def h2s(k, c):
                    return h2q[(k // 4, c)][:, (k % 4) * CW:(k % 4 + 1) * CW]

                m_t = [h2p.tile([128, S], F16, tag=f"mm{i}", name=f"mres{i}")
                       for i in range(IL_T)]
                with (
                    tc.tile_pool(name=P + "p5", bufs=2) as p5,
                    tc.tile_pool(name=P + "p5m", bufs=2) as p5m,
                    tc.tile_pool(name=P + "p5ps", bufs=1, space="PSUM") as p5ps,
                    tc.tile_pool(name=P + "ugw", bufs=2) as ugw,
                    tc.tile_pool(name=P + "ugps", bufs=2, space="PSUM") as ugps,
                    tc.tile_pool(name=P + "ugt", bufs=3) as ugt,
                    tc.tile_pool(name=P + "dw", bufs=6) as dwp,
                    tc.tile_pool(name=P + "dps", bufs=2, space="PSUM") as dps,
                    tc.tile_pool(name=P + "dev", bufs=4) as dev,
                ):
                    # h2 chunk loads (gpsimd queue; each waits on AR chunk c)
                    for c in range(CH):
                        for kq in range(KT // 4):
                            nc.gpsimd.dma_start(
                                h2q[(kq, c)][:].rearrange(
                                    "p (j w) -> p j w", j=4),
                                h2d[c][kq * 512:(kq + 1) * 512, :].rearrange(
                                    "(j p) w -> p j w", p=128))

                    def emit_upgate(c):
                        cs = slice(c * CW, (c + 1) * CW)
                        # stats for this chunk
                        r2 = p5ps.tile([128, CW], F32, tag=f"r2b{c}",
                                       name=f"r2b{c}")
                        for k in range(KT):
                            sq = p5.tile([128, CW], BF16, tag="sq")
                            nc.vector.tensor_mul(sq[:], h2s(k, c), h2s(k, c))
                            nc.tensor.matmul(r2[:], onesb[:], sq[:],
                                             start=(k == 0), stop=(k == KT - 1))
                        ms = p5m.tile([128, CW], F32, tag="ms")
                        nc.scalar.activation(ms[:], r2[:], AF.Sqrt,
                                             bias=epst[:], scale=1.0 / HID)
                        nc.vector.reciprocal(s2[:, cs], ms[:])
                        # up/gate for this chunk (s2 applied at evac);
                        # u+g slabs land with a single fused DMA per d
                        for d in range(IL_T):
                            sl = ugw.tile([128, 2 * HID], F16, tag="sl",
                                          name="slab")
                            nc.sync.dma_start(
                                sl[:].rearrange("p (g h) -> p g h", g=2),
                                wug[d, :, :, :].rearrange("g p h -> p g h"))
                            pts = []
                            for g in range(2):
                                pt = ugps.tile([128, CW], F32, tag=f"pt{g}",
                                               name=f"ptug{g}")
                                for k in range(KT):
                                    nc.tensor.matmul(
                                        pt[:],
                                        sl[:, g * HID + k * 128:
                                           g * HID + (k + 1) * 128],
                                        h2s(k, c),
                                        start=(k == 0), stop=(k == KT - 1))
                                pts.append(pt)
                            un = ugt.tile([128, CW], F32, tag="un")
                            nc.vector.tensor_mul(un[:], pts[0][:], s2[:, cs])
                            sil = ugt.tile([128, CW], F32, tag="sil")
                            nc.scalar.activation(sil[:], un[:], AF.Silu)
                            gn = ugt.tile([128, CW], F32, tag="gn")
                            nc.vector.tensor_mul(gn[:], pts[1][:], s2[:, cs])
                            nc.vector.tensor_mul(m_t[d][:, cs], sil[:], gn[:])

                    def emit_down(j):
                        # down-proj + h2/8 fold -> AR per chunk + output copy
                        c, r0, r1 = DN_CHUNKS[j]
                        cs = slice(c * CW, (c + 1) * CW)
                        for mh in range(r0 // 128, r1 // 128):
                            sl = dwp.tile([128, IL], F16, tag="dw")
                            nc.sync.dma_start(sl[:], wdn[mh, :, :])
                            pt = dps.tile([128, CW], F32, tag="pt")
                            for i in range(IL_T):
                                nc.tensor.matmul(
                                    pt[:], sl[:, i * 128:(i + 1) * 128],
                                    m_t[i][:, cs],
                                    start=(i == 0), stop=(i == IL_T - 1))
                            ev = dev.tile([128, CW], F16, tag="ev")
                            nc.vector.scalar_tensor_tensor(
                                ev[:], h2s(mh, c), 1.0 / N_CORES, pt[:],
                                op0=ALU.mult, op1=ALU.add)
                            nc.gpsimd.dma_start(
                                dnb[j][mh * 128 - r0:(mh + 1) * 128 - r0, :],
                                ev[:])
                        all_reduce(dnr[j], dnb[j])
                        nc.gpsimd.dma_start(
                            outT[r0:r1, c * CW:(c + 1) * CW], dnr[j][:])

                    # down-c0 right after upgate-c0: its AR chunks drain on
                    # the CC stream while upgate-c1 computes.
                    emit_upgate(0)
                    emit_down(0)
                    emit_down(1)
                    emit_upgate(1)
                    emit_down(2)
                    emit_down(3)
                    emit_down(4)

    nc.compile()
    return nc


def _host_shard(hidden_states, mask, wq, wk, wv, wo, w_gate, w_up, w_down,
                g_in, g_post):
    import ml_dtypes
    x = np.asarray(hidden_states, dtype=np.float32).reshape(S, HID)
    xT = np.ascontiguousarray(x.T).astype(np.float16)
    maskT = np.ascontiguousarray(np.asarray(mask, dtype=np.float32)
                                 .reshape(S, S).T)
    maskTd = np.empty((TB, 128, CW), np.float32)
    for t in range(TB):
        c = t // (TB // CH)
        maskTd[t] = maskT[t * 128:(t + 1) * 128, c * CW:(c + 1) * CW]
    maskTd = maskTd.astype(ml_dtypes.bfloat16)
    g_in = np.asarray(g_in, dtype=np.float32)
    g_post = np.asarray(g_post, dtype=np.float32)
    sc = np.float32(HD ** -0.5)

    in_maps = []
    for i in range(N_CORES):
        r0, r1 = i * DL, (i + 1) * DL
        i0, i1 = i * ILR, (i + 1) * ILR
        wqT = (wq[r0:r1].T * (g_in[:, None] * sc)).astype(np.float16)
        wkT = (wk[r0:r1].T * g_in[:, None]).astype(np.float16)
        wvT = (wv[r0:r1].T * g_in[:, None]).astype(np.float16)
        woT = np.ascontiguousarray(wo[:, r0:r1].T).astype(np.float16)
        # up/gate slabs: wug[d, g, p, k*128+j] = w[i0+d*128+j, k*128+p]*g_post
        wug = np.zeros((IL_T, 2, 128, HID), np.float16)
        for g, w in ((0, w_up), (1, w_gate)):
            wl = (w[i0:i1].T * g_post[:, None]).astype(np.float32)  # [HID,ILR]
            wl = np.pad(wl, ((0, 0), (0, IL - ILR)))
            # [HID, IL] -> [KT,128,IL_T,128] -> [IL_T, 128(p=k-row), KT*128]
            wug[:, g] = (wl.reshape(KT, 128, IL_T, 128)
                         .transpose(2, 1, 0, 3).reshape(IL_T, 128, HID)
                         .astype(np.float16))
        # down slabs: wdn[mh, p, i*128+j] = w_down[mh*128+j, i0+i*128+p]
        wdl = np.pad(w_down[:, i0:i1].T.astype(np.float32),
                     ((0, IL - ILR), (0, 0)))      # [IL, HID]
        wdn = (wdl.reshape(IL_T, 128, KT, 128)
               .transpose(2, 1, 0, 3).reshape(KT, 128, IL)
               .astype(np.float16))
        in_maps.append({
            "xT": xT, "maskTd": maskTd, "wqT": wqT, "wkT": wkT, "wvT": wvT,
            "woT": woT, "wug": wug, "wdn": wdn,
        })
    return in_maps


def _get_nc(repeat=1):
    key = ("nc", repeat)
    if key not in _CACHE:
        _CACHE[key] = _build(repeat=repeat)
    return _CACHE[key]


def kernel(**inputs):
    nc = _get_nc()
    in_maps = _host_shard(**{k: np.asarray(v) for k, v in inputs.items()})
    res = run_bass_kernel_spmd(nc, in_maps, list(range(N_CORES)))
    outT = res.results[0]["outT"].astype(np.float32)
    return np.ascontiguousarray(outT.T).reshape(1, S, HID)


def _make_runner(repeat=1, **inputs):
    """Build the compiled sharded callable + device-resident inputs once.
    Returns run() -> (wall_ns, outs)."""
    import time
    import jax
    from jax.sharding import Mesh, PartitionSpec
    from jax.experimental.shard_map import shard_map
    from concourse import bass2jax

    nc = _get_nc(repeat)
    in_maps = _host_shard(**{k: np.asarray(v) for k, v in inputs.items()})
    bass2jax.install_neuronx_cc_hook()

    partition_name = (nc.partition_id_tensor.name
                      if nc.partition_id_tensor else None)
    in_names, out_names, out_avals, zero_outs = [], [], [], []
    for alloc in nc.m.functions[0].allocations:
        if not isinstance(alloc, mybir.MemoryLocationSet):
            continue
        name = alloc.memorylocations[0].name
        if alloc.kind == "ExternalInput":
            if name != partition_name:
                in_names.append(name)
        elif alloc.kind == "ExternalOutput":
            out_names.append(name)
            shape = tuple(alloc.tensor_shape)
            dtype = mybir.dt.np(alloc.dtype)
            out_avals.append(jax.core.ShapedArray(shape, dtype))
            zero_outs.append(np.zeros(shape, dtype))
    n_params = len(in_names)
    all_in = list(in_names) + list(out_names)
    if partition_name is not None:
        all_in.append(partition_name)

    def _body(*args):
        operands = list(args)
        if partition_name is not None:
            operands.append(bass2jax.partition_id_tensor())
        outs = bass2jax._bass_exec_p.bind(
            *operands,
            out_avals=tuple(out_avals), in_names=tuple(all_in),
            out_names=tuple(out_names), lowering_input_output_aliases=(),
            sim_require_finite=True, sim_require_nnan=True, nc=nc)
        return tuple(outs)

    devices = jax.devices()[:N_CORES]
    mesh = Mesh(np.asarray(devices), ("core",))
    n_outs = len(out_names)
    in_specs = (PartitionSpec("core"),) * (n_params + n_outs)
    out_specs = (PartitionSpec("core"),) * n_outs
    fn = jax.jit(shard_map(_body, mesh=mesh, in_specs=in_specs,
                           out_specs=out_specs, check_rep=False))
    concat_in = [np.concatenate([np.asarray(in_maps[c][nm])
                                 for c in range(N_CORES)], axis=0)
                 for nm in in_names]
    concat_zeros = [np.zeros((N_CORES * z.shape[0], *z.shape[1:]), z.dtype)
                    for z in zero_outs]
    sharding = jax.sharding.NamedSharding(mesh, PartitionSpec("core"))
    dev_in = [jax.device_put(a, sharding) for a in concat_in]
    dev_zero = [jax.device_put(a, sharding) for a in concat_zeros]

    outs = fn(*dev_in, *dev_zero)          # warm-up / compile
    jax.block_until_ready(outs)

    def run():
        t0 = time.perf_counter_ns()
        o = fn(*dev_in, *dev_zero)
        jax.block_until_ready(o)
        return time.perf_counter_ns() - t0, o

    def unpack(o):
        return {nm: np.asarray(o[i]).reshape(N_CORES, *out_avals[i].shape)[0]
                for i, nm in enumerate(out_names)}

    return run, unpack


def bench(iters=8, repeat=1, **inputs):
    """Time repeated on-device executions; returns (best_ns, core0 outputs)."""
    run, unpack = _make_runner(repeat=repeat, **inputs)
    best, outs = float("inf"), None
    for _ in range(iters):
        ns, outs = run()
        best = min(best, ns)
    return best, unpack(outs)


# revision 19
# speedup vs baseline: 1.0254x; 1.0254x over previous
"""Mistral decoder layer (B=1, S=1024, HID=4096, 32 heads, INTER=11008),
tensor-parallel over 8 trn2 NeuronCores (Megatron style).

v2 redesign (from NTFF trace analysis of the fp32 baseline):
  - PE clock is throttled to ~1.2-1.7GHz effective regardless of dtype, so
    matmul time is fixed; the wins are overlap + memory:
  - f16 storage/operands everywhere (psum accumulation stays fp32):
    halves weight DMA (101MB -> 51MB/core), keeps x resident (no 16MB
    reload at o-proj), h2 consumed directly from the f16 AllReduce (no
    upcast), m held in SBUF (no DRAM bounce). et/V use bf16 (exp range).
  - host pre-lays-out every weight so each DMA is >=1KB-contiguous rows.
  - attention(c) interleaved with o-proj(c); o-proj AllReduce chunked per
    column-half (2 x 4MB) so AR(c0) hides under attn(c1)+o(c1) and AR(c1)
    under up/gate(c0); down AllReduce chunked 4 x 2MB to shrink the tail.
  - RMSNorm stats via ones-matmuls (fp32r), gains + 1/sqrt(hd) folded into
    weights on host; residuals folded into the AllReduces (x/8, h2/8).
"""

import numpy as np

import concourse.bacc as bacc
import concourse.mybir as mybir
import concourse.tile as tile
from concourse.bass_utils import run_bass_kernel_spmd

AF = mybir.ActivationFunctionType
ALU = mybir.AluOpType
F32 = mybir.dt.float32
F32R = mybir.dt.float32r
F16 = mybir.dt.float16
BF16 = mybir.dt.bfloat16

N_CORES = 8
HID = 4096
S = 1024
NH = 32
HD = 128
NH_L = NH // N_CORES          # 4 local heads
DL = NH_L * HD                # 512 local q/k/v dims
INTER = 11008
IL_T = 11                     # local intermediate k-tiles (padded)
IL = IL_T * 128               # 1408 padded local intermediate
ILR = INTER // N_CORES        # 1376 real local intermediate
KT = HID // 128               # 32 hidden k-tiles
CH = 2                        # seq chunks
CW = S // CH                  # 512
TB = S // 128                 # 8 seq tiles of 128
EPS = 1e-5

_CACHE = {}


def _r(ap):
    return ap.bitcast(F32R)


def _build(collectives=True, repeat=1):
    nc = bacc.Bacc("TRN2", target_bir_lowering=False, debug=False,
                   num_devices=N_CORES)

    xT = nc.dram_tensor("xT", [HID, S], F16, kind="ExternalInput").ap()
    maskTd = nc.dram_tensor("maskTd", [TB, 128, CW], BF16,
                            kind="ExternalInput").ap()
    wqT = nc.dram_tensor("wqT", [HID, DL], F16, kind="ExternalInput").ap()
    wkT = nc.dram_tensor("wkT", [HID, DL], F16, kind="ExternalInput").ap()
    wvT = nc.dram_tensor("wvT", [HID, DL], F16, kind="ExternalInput").ap()
    woT = nc.dram_tensor("woT", [DL, HID], F16, kind="ExternalInput").ap()
    wug = nc.dram_tensor("wug", [IL_T, 2, 128, HID], F16,
                         kind="ExternalInput").ap()
    wdn = nc.dram_tensor("wdn", [KT, 128, IL], F16, kind="ExternalInput").ap()
    outT = nc.dram_tensor("outT", [HID, S], F16, kind="ExternalOutput").ap()

    # o-proj AR: one chunk per column-half c: [HID, CW] f16 (4MB)
    ob = [nc.dram_tensor(f"ob{c}", [HID, CW], F16).ap() for c in range(CH)]
    h2d = [nc.dram_tensor(f"h2d{c}", [HID, CW], F16, addr_space="Shared").ap()
           for c in range(CH)]
    s1_d = nc.dram_tensor("s1_d", [S], F32).ap()
    # down AR chunks: (row-range, col-chunk) pairs; trailing chunks are
    # smaller (1MB) to shrink the tail-AR exposure.
    DN_CHUNKS = [  # (c, row0, row1)
        (0, 0, 2048), (0, 2048, 4096), (1, 0, 2048),
        (1, 2048, 3072), (1, 3072, 4096),
    ]
    dnb = [nc.dram_tensor(f"dnb{j}", [r1 - r0, CW], F16).ap()
           for j, (c, r0, r1) in enumerate(DN_CHUNKS)]
    dnr = [nc.dram_tensor(f"dnr{j}", [r1 - r0, CW], F16,
                          addr_space="Shared").ap()
           for j, (c, r0, r1) in enumerate(DN_CHUNKS)]

    rg = [list(range(N_CORES))]

    def all_reduce(dst, srcs):
        if collectives:
            nc.gpsimd.collective_compute(
                "AllReduce", ALU.add, ins=[srcs[:]], outs=[dst[:]],
                replica_groups=rg)
        else:
            nc.gpsimd.dma_start(dst[:], srcs[:])

    with tile.TileContext(nc) as tc:
      for rep in range(repeat):
        P = f"r{rep}_" if repeat > 1 else ""
        with tc.tile_pool(name=P + "const", bufs=1) as const:
            onesb = const.tile([128, 128], BF16, tag="onesb")
            nc.vector.memset(onesb[:], 1.0)
            s1 = const.tile([128, S], F32, tag="s1")
            s1t = const.tile([128, TB], F32, tag="s1t")
            s2 = const.tile([128, S], F32, tag="s2")
            epst = const.tile([128, 1], F32, tag="epst")
            nc.vector.memset(epst[:], EPS)

            with tc.tile_pool(name=P + "qkvo", bufs=1) as qkvo:
                QTt = [qkvo.tile([128, S], F16, tag=f"QT{h}", name=f"QT{h}")
                       for h in range(NH_L)]
                KTt = [qkvo.tile([128, S], F16, tag=f"KT{h}", name=f"KT{h}")
                       for h in range(NH_L)]
                Vt = [qkvo.tile([128, DL], BF16, tag=f"V{t}", name=f"V{t}")
                      for t in range(TB)]
                ATt = [qkvo.tile([128, S], F16, tag=f"AT{h}", name=f"AT{h}")
                       for h in range(NH_L)]
                wo_r = [qkvo.tile([128, HID], F16, tag=f"wo{h}", name=f"wo{h}")
                        for h in range(NH_L)]
                mtiles = [qkvo.tile([128, CW], BF16, tag=f"m{t}", name=f"mk{t}")
                          for t in range(TB)]

                with tc.tile_pool(name=P + "xres", bufs=1) as xres:
                    xt = [xres.tile([128, S], F16, tag=f"x{k}", name=f"x{k}")
                          for k in range(KT)]
                    # ---- phase 0: x load + RMSNorm#1 stats ----
                    with (
                        tc.tile_pool(name=P + "p0", bufs=2) as p0,
                        tc.tile_pool(name=P + "p0m", bufs=2) as p0m,
                        tc.tile_pool(name=P + "p0ps", bufs=1, space="PSUM") as p0ps,
                    ):
                        r2 = [p0ps.tile([128, CW], F32, tag=f"r2_{c}",
                                        name=f"r2_{c}") for c in range(CH)]
                        for k in range(KT):
                            eng = nc.sync if k % 2 == 0 else nc.gpsimd
                            eng.dma_start(xt[k][:],
                                          xT[k * 128:(k + 1) * 128, :])
                            sq = p0.tile([128, S], BF16, tag="sq", name=f"sq{k}")
                            nc.vector.tensor_mul(sq[:], xt[k][:], xt[k][:])
                            for c in range(CH):
                                nc.tensor.matmul(
                                    r2[c][:], onesb[:],
                                    sq[:, c * CW:(c + 1) * CW],
                                    start=(k == 0), stop=(k == KT - 1))
                        for c in range(CH):
                            ms = p0m.tile([128, CW], F32, tag="ms")
                            nc.scalar.activation(ms[:], r2[c][:], AF.Sqrt,
                                                 bias=epst[:], scale=1.0 / HID)
                            nc.vector.reciprocal(s1[:, c * CW:(c + 1) * CW],
                                                 ms[:])
                    # mask + resident wo load on the gpsimd queue (ahead of
                    # the s1 bounce, which waits on the stats)
                    for t in range(TB):
                        nc.gpsimd.dma_start(mtiles[t][:], maskTd[t, :, :])
                    for h in range(NH_L):
                        nc.gpsimd.dma_start(wo_r[h][:],
                                            woT[h * 128:(h + 1) * 128, :])
                    # s1t = s1 transposed down partitions, via a DRAM bounce
                    nc.gpsimd.dma_start(s1_d.rearrange("(o s) -> o s", o=1),
                                        s1[0:1, :])
                    nc.gpsimd.dma_start(s1t[:],
                                        s1_d.rearrange("(t p) -> p t", p=128))

                    # ---- phase 1: q/k passes (weights stream) ----
                    for nm, wT, outs in (("q", wqT, QTt), ("k", wkT, KTt)):
                        with (
                            tc.tile_pool(name=P + f"{nm}w", bufs=3) as wp,
                            tc.tile_pool(name=P + f"{nm}ps", bufs=1,
                                         space="PSUM") as ps,
                        ):
                            pt = [ps.tile([128, CW], F32, tag=f"pt{j}",
                                          name=f"pt{j}") for j in range(NH_L * CH)]
                            for k in range(KT):
                                wt = wp.tile([128, DL], F16, tag="wt")
                                nc.sync.dma_start(
                                    wt[:], wT[k * 128:(k + 1) * 128, :])
                                for h in range(NH_L):
                                    for c in range(CH):
                                        nc.tensor.matmul(
                                            pt[h * CH + c][:],
                                            wt[:, h * 128:(h + 1) * 128],
                                            xt[k][:, c * CW:(c + 1) * CW],
                                            start=(k == 0), stop=(k == KT - 1))
                            for h in range(NH_L):
                                for c in range(CH):
                                    nc.vector.tensor_mul(
                                        outs[h][:, c * CW:(c + 1) * CW],
                                        pt[h * CH + c][:],
                                        s1[:, c * CW:(c + 1) * CW])

                    # v pass: V[t] rows scaled by s1t column
                    with (
                        tc.tile_pool(name=P + "vw", bufs=3) as wp,
                        tc.tile_pool(name=P + "vps", bufs=1, space="PSUM") as ps,
                    ):
                        pt = [ps.tile([128, DL], F32, tag=f"pt{t}", name=f"pt{t}")
                              for t in range(TB)]
                        for k in range(KT):
                            wt = wp.tile([128, DL], F16, tag="wt")
                            nc.sync.dma_start(
                                wt[:], wvT[k * 128:(k + 1) * 128, :])
                            for t in range(TB):
                                nc.tensor.matmul(
                                    pt[t][:], xt[k][:, t * 128:(t + 1) * 128],
                                    wt[:], start=(k == 0), stop=(k == KT - 1))
                        for t in range(TB):
                            nc.vector.tensor_scalar(
                                Vt[t][:], pt[t][:], s1t[:, t:t + 1], None,
                                op0=ALU.mult)

                    # ---- phases 2+3 interleaved per column-half c:
                    #      attention(c) then o-proj(c) -> AR chunk c ----
                    with (
                        tc.tile_pool(name=P + "est", bufs=3) as estp,
                        tc.tile_pool(name=P + "etm", bufs=1) as etmp,
                        tc.tile_pool(name=P + "rin", bufs=2) as rinp,
                        tc.tile_pool(name=P + "aps", bufs=1, space="PSUM") as aps,
                        tc.tile_pool(name=P + "stps", bufs=3, space="PSUM") as stps,
                        tc.tile_pool(name=P + "ops", bufs=2, space="PSUM") as ops,
                        tc.tile_pool(name=P + "oev", bufs=4) as oev,
                    ):
                        atp = [aps.tile([128, CW], F32, tag=f"atp{j}",
                                        name=f"atp{j}") for j in range(2)]
                        rsp = aps.tile([128, CW], F32, tag="rsp", name="rsp")
                        # mask -> exp(mask) in place: masked et becomes a
                        # cheap bf16 multiply, and exp always reads PSUM
                        # directly (no DVE add on the exp critical path).
                        for t in range(TB):
                            nc.scalar.activation(mtiles[t][:], mtiles[t][:],
                                                 AF.Exp)

                        def emit_st(c, h, ets):
                            cs = slice(c * CW, (c + 1) * CW)
                            for t in range(0, (c + 1) * 4):
                                stp = stps.tile([128, CW], F32, tag="st")
                                nc.tensor.matmul(
                                    stp[:], KTt[h][:, t * 128:(t + 1) * 128],
                                    QTt[h][:, cs], start=True, stop=True)
                                et = etmp.tile([128, CW], BF16,
                                               tag=f"et{h}_{t}",
                                               name=f"et{h}_{t}")
                                if t >= c * 4:
                                    es = estp.tile([128, CW], BF16, tag="es")
                                    nc.scalar.activation(es[:], stp[:], AF.Exp)
                                    nc.vector.tensor_mul(et[:], es[:],
                                                         mtiles[t][:])
                                else:
                                    nc.scalar.activation(et[:], stp[:], AF.Exp)
                                ets[(h, t)] = et

                        def emit_pv(c, h, ets):
                            cs = slice(c * CW, (c + 1) * CW)
                            tbs = list(range(0, (c + 1) * 4))
                            ap_ = atp[h % 2]
                            for j, t in enumerate(tbs):
                                st_, sp_ = (j == 0), (j == len(tbs) - 1)
                                nc.tensor.matmul(
                                    ap_[:], Vt[t][:, h * 128:(h + 1) * 128],
                                    ets[(h, t)][:], start=st_, stop=sp_)
                                nc.tensor.matmul(
                                    rsp[:], onesb[:], ets[(h, t)][:],
                                    start=st_, stop=sp_)
                            ri = rinp.tile([128, CW], F32, tag="ri")
                            nc.vector.reciprocal(ri[:], rsp[:])
                            nc.vector.tensor_mul(ATt[h][:, cs], ap_[:], ri[:])

                        for c in range(CH):
                            cs = slice(c * CW, (c + 1) * CW)
                            # software-pipelined: STs run ahead of PV chains
                            # so exp/mul latency hides under tensor work.
                            ets = {}
                            emit_st(c, 0, ets)
                            emit_st(c, 1, ets)
                            emit_st(c, 2, ets)
                            emit_pv(c, 0, ets)
                            emit_st(c, 3, ets)
                            emit_pv(c, 1, ets)
                            emit_pv(c, 2, ets)
                            emit_pv(c, 3, ets)
                            # o-proj for this column-half + x/8 fold
                            for mh in range(KT):
                                pt = ops.tile([128, CW], F32, tag="pt")
                                for h in range(NH_L):
                                    nc.tensor.matmul(
                                        pt[:],
                                        wo_r[h][:, mh * 128:(mh + 1) * 128],
                                        ATt[h][:, cs],
                                        start=(h == 0), stop=(h == NH_L - 1))
                                ev = oev.tile([128, CW], F16, tag="ev")
                                nc.vector.scalar_tensor_tensor(
                                    ev[:], xt[mh][:, cs], 1.0 / N_CORES, pt[:],
                                    op0=ALU.mult, op1=ALU.add)
                                nc.gpsimd.dma_start(
                                    ob[c][mh * 128:(mh + 1) * 128, :], ev[:])
                            all_reduce(h2d[c], ob[c])

            # ---- phases 4+5: RMSNorm#2 stats + up/gate per column-half,
            #      then down-proj + chunked AR ----
            with tc.tile_pool(name=P + "h2res", bufs=1) as h2p:
                # h2 held as quad tiles [128, 4*CW] per (kq, c): one DMA per
                # quad (4x fewer dispatches on the AR->MLP critical path)
                h2q = {}
                for kq in range(KT // 4):
                    for c in range(CH):
                        h2q[(kq, c)] = h2p.tile([128, 4 * CW], F16,
                                                tag=f"h2q{kq}_{c}",
                                                name=f"h2q{kq}_{c}")

                def h2s(k, c):
                    return h2q[(k // 4, c)][:, (k % 4) * CW:(k % 4 + 1) * CW]

                m_t = [h2p.tile([128, S], F16, tag=f"mm{i}", name=f"mres{i}")
                       for i in range(IL_T)]
                with (
                    tc.tile_pool(name=P + "p5", bufs=2) as p5,
                    tc.tile_pool(name=P + "p5m", bufs=2) as p5m,
                    tc.tile_pool(name=P + "p5ps", bufs=1, space="PSUM") as p5ps,
                    tc.tile_pool(name=P + "ugw", bufs=3) as ugw,
                    tc.tile_pool(name=P + "ugps", bufs=2, space="PSUM") as ugps,
                    tc.tile_pool(name=P + "ugt", bufs=3) as ugt,
                    tc.tile_pool(name=P + "dw", bufs=8) as dwp,
                    tc.tile_pool(name=P + "dps", bufs=2, space="PSUM") as dps,
                    tc.tile_pool(name=P + "dev", bufs=4) as dev,
                ):
                    # h2 chunk loads (gpsimd queue; each waits on AR chunk c)
                    for c in range(CH):
                        for kq in range(KT // 4):
                            nc.gpsimd.dma_start(
                                h2q[(kq, c)][:].rearrange(
                                    "p (j w) -> p j w", j=4),
                                h2d[c][kq * 512:(kq + 1) * 512, :].rearrange(
                                    "(j p) w -> p j w", p=128))

                    def emit_upgate(c):
                        cs = slice(c * CW, (c + 1) * CW)
                        # stats for this chunk
                        r2 = p5ps.tile([128, CW], F32, tag=f"r2b{c}",
                                       name=f"r2b{c}")
                        for k in range(KT):
                            sq = p5.tile([128, CW], BF16, tag="sq")
                            nc.vector.tensor_mul(sq[:], h2s(k, c), h2s(k, c))
                            nc.tensor.matmul(r2[:], onesb[:], sq[:],
                                             start=(k == 0), stop=(k == KT - 1))
                        ms = p5m.tile([128, CW], F32, tag="ms")
                        nc.scalar.activation(ms[:], r2[:], AF.Sqrt,
                                             bias=epst[:], scale=1.0 / HID)
                        nc.vector.reciprocal(s2[:, cs], ms[:])
                        # up/gate for this chunk (s2 applied at evac);
                        # u+g slabs land with a single fused DMA per d
                        for d in range(IL_T):
                            sl = ugw.tile([128, 2 * HID], F16, tag="sl",
                                          name="slab")
                            nc.sync.dma_start(
                                sl[:].rearrange("p (g h) -> p g h", g=2),
                                wug[d, :, :, :].rearrange("g p h -> p g h"))
                            pts = []
                            for g in range(2):
                                pt = ugps.tile([128, CW], F32, tag=f"pt{g}",
                                               name=f"ptug{g}")
                                for k in range(KT):
                                    nc.tensor.matmul(
                                        pt[:],
                                        sl[:, g * HID + k * 128:
                                           g * HID + (k + 1) * 128],
                                        h2s(k, c),
                                        start=(k == 0), stop=(k == KT - 1))
                                pts.append(pt)
                            un = ugt.tile([128, CW], F32, tag="un")
                            nc.vector.tensor_mul(un[:], pts[0][:], s2[:, cs])
                            sil = ugt.tile([128, CW], F32, tag="sil")
                            nc.scalar.activation(sil[:], un[:], AF.Silu)
                            gn = ugt.tile([128, CW], F32, tag="gn")
                            nc.vector.tensor_mul(gn[:], pts[1][:], s2[:, cs])
                            nc.vector.tensor_mul(m_t[d][:, cs], sil[:], gn[:])

                    def emit_down(j):
                        # down-proj + h2/8 fold -> AR per chunk + output copy
                        c, r0, r1 = DN_CHUNKS[j]
                        cs = slice(c * CW, (c + 1) * CW)
                        for mh in range(r0 // 128, r1 // 128):
                            sl = dwp.tile([128, IL], F16, tag="dw")
                            nc.sync.dma_start(sl[:], wdn[mh, :, :])
                            pt = dps.tile([128, CW], F32, tag="pt")
                            for i in range(IL_T):
                                nc.tensor.matmul(
                                    pt[:], sl[:, i * 128:(i + 1) * 128],
                                    m_t[i][:, cs],
                                    start=(i == 0), stop=(i == IL_T - 1))
                            ev = dev.tile([128, CW], F16, tag="ev")
                            nc.vector.scalar_tensor_tensor(
                                ev[:], h2s(mh, c), 1.0 / N_CORES, pt[:],
                                op0=ALU.mult, op1=ALU.add)
                            nc.gpsimd.dma_start(
                                dnb[j][mh * 128 - r0:(mh + 1) * 128 - r0, :],
                                ev[:])
                        all_reduce(dnr[j], dnb[j])
                        nc.gpsimd.dma_start(
                            outT[r0:r1, c * CW:(c + 1) * CW], dnr[j][:])

                    # down-c0 right after upgate-c0: its AR chunks drain on
                    # the CC stream while upgate-c1 computes.
                    emit_upgate(0)
                    emit_down(0)
                    emit_down(1)
                    emit_upgate(1)
                    emit_down(2)
                    emit_down(3)
                    emit_down(4)

    nc.compile()
    return nc


def _host_shard(hidden_states, mask, wq, wk, wv, wo, w_gate, w_up, w_down,
                g_in, g_post):
    import ml_dtypes
    x = np.asarray(hidden_states, dtype=np.float32).reshape(S, HID)
    xT = np.ascontiguousarray(x.T).astype(np.float16)
    maskT = np.ascontiguousarray(np.asarray(mask, dtype=np.float32)
                                 .reshape(S, S).T)
    maskTd = np.empty((TB, 128, CW), np.float32)
    for t in range(TB):
        c = t // (TB // CH)
        maskTd[t] = maskT[t * 128:(t + 1) * 128, c * CW:(c + 1) * CW]
    maskTd = maskTd.astype(ml_dtypes.bfloat16)
    g_in = np.asarray(g_in, dtype=np.float32)
    g_post = np.asarray(g_post, dtype=np.float32)
    sc = np.float32(HD ** -0.5)

    in_maps = []
    for i in range(N_CORES):
        r0, r1 = i * DL, (i + 1) * DL
        i0, i1 = i * ILR, (i + 1) * ILR
        wqT = (wq[r0:r1].T * (g_in[:, None] * sc)).astype(np.float16)
        wkT = (wk[r0:r1].T * g_in[:, None]).astype(np.float16)
        wvT = (wv[r0:r1].T * g_in[:, None]).astype(np.float16)
        woT = np.ascontiguousarray(wo[:, r0:r1].T).astype(np.float16)
        # up/gate slabs: wug[d, g, p, k*128+j] = w[i0+d*128+j, k*128+p]*g_post
        wug = np.zeros((IL_T, 2, 128, HID), np.float16)
        for g, w in ((0, w_up), (1, w_gate)):
            wl = (w[i0:i1].T * g_post[:, None]).astype(np.float32)  # [HID,ILR]
            wl = np.pad(wl, ((0, 0), (0, IL - ILR)))
            # [HID, IL] -> [KT,128,IL_T,128] -> [IL_T, 128(p=k-row), KT*128]
            wug[:, g] = (wl.reshape(KT, 128, IL_T, 128)
                         .transpose(2, 1, 0, 3).reshape(IL_T, 128, HID)
                         .astype(np.float16))
        # down slabs: wdn[mh, p, i*128+j] = w_down[mh*128+j, i0+i*128+p]
        wdl = np.pad(w_down[:, i0:i1].T.astype(np.float32),
                     ((0, IL - ILR), (0, 0)))      # [IL, HID]
        wdn = (wdl.reshape(IL_T, 128, KT, 128)
               .transpose(2, 1, 0, 3).reshape(KT, 128, IL)
               .astype(np.float16))
        in_maps.append({
            "xT": xT, "maskTd": maskTd, "wqT": wqT, "wkT": wkT, "wvT": wvT,
            "woT": woT, "wug": wug, "wdn": wdn,
        })
    return in_maps


def _get_nc(repeat=1):
    key = ("nc", repeat)
    if key not in _CACHE:
        _CACHE[key] = _build(repeat=repeat)
    return _CACHE[key]


def kernel(**inputs):
    nc = _get_nc()
    in_maps = _host_shard(**{k: np.asarray(v) for k, v in inputs.items()})
    res = run_bass_kernel_spmd(nc, in_maps, list(range(N_CORES)))
    outT = res.results[0]["outT"].astype(np.float32)
    return np.ascontiguousarray(outT.T).reshape(1, S, HID)


def _make_runner(repeat=1, **inputs):
    """Build the compiled sharded callable + device-resident inputs once.
    Returns run() -> (wall_ns, outs)."""
    import time
    import jax
    from jax.sharding import Mesh, PartitionSpec
    from jax.experimental.shard_map import shard_map
    from concourse import bass2jax

    nc = _get_nc(repeat)
    in_maps = _host_shard(**{k: np.asarray(v) for k, v in inputs.items()})
    bass2jax.install_neuronx_cc_hook()

    partition_name = (nc.partition_id_tensor.name
                      if nc.partition_id_tensor else None)
    in_names, out_names, out_avals, zero_outs = [], [], [], []
    for alloc in nc.m.functions[0].allocations:
        if not isinstance(alloc, mybir.MemoryLocationSet):
            continue
        name = alloc.memorylocations[0].name
        if alloc.kind == "ExternalInput":
            if name != partition_name:
                in_names.append(name)
        elif alloc.kind == "ExternalOutput":
            out_names.append(name)
            shape = tuple(alloc.tensor_shape)
            dtype = mybir.dt.np(alloc.dtype)
            out_avals.append(jax.core.ShapedArray(shape, dtype))
            zero_outs.append(np.zeros(shape, dtype))
    n_params = len(in_names)
    all_in = list(in_names) + list(out_names)
    if partition_name is not None:
        all_in.append(partition_name)

    def _body(*args):
        operands = list(args)
        if partition_name is not None:
            operands.append(bass2jax.partition_id_tensor())
        outs = bass2jax._bass_exec_p.bind(
            *operands,
            out_avals=tuple(out_avals), in_names=tuple(all_in),
            out_names=tuple(out_names), lowering_input_output_aliases=(),
            sim_require_finite=True, sim_require_nnan=True, nc=nc)
        return tuple(outs)

    devices = jax.devices()[:N_CORES]
    mesh = Mesh(np.asarray(devices), ("core",))
    n_outs = len(out_names)
    in_specs = (PartitionSpec("core"),) * (n_params + n_outs)
    out_specs = (PartitionSpec("core"),) * n_outs
    fn = jax.jit(shard_map(_body, mesh=mesh, in_specs=in_specs,
                           out_specs=out_specs, check_rep=False))
    concat_in = [np.concatenate([np.asarray(in_maps[c][nm])
                                 for c in range(N_CORES)], axis=0)
                 for nm in in_names]
    concat_zeros = [np.zeros((N_CORES * z.shape[0], *z.shape[1:]), z.dtype)
                    for z in zero_outs]
    sharding = jax.sharding.NamedSharding(mesh, PartitionSpec("core"))
    dev_in = [jax.device_put(a, sharding) for a in concat_in]
    dev_zero = [jax.device_put(a, sharding) for a in concat_zeros]

    outs = fn(*dev_in, *dev_zero)          # warm-up / compile
    jax.block_until_ready(outs)

    def run():
        t0 = time.perf_counter_ns()
        o = fn(*dev_in, *dev_zero)
        jax.block_until_ready(o)
        return time.perf_counter_ns() - t0, o

    def unpack(o):
        return {nm: np.asarray(o[i]).reshape(N_CORES, *out_avals[i].shape)[0]
                for i, nm in enumerate(out_names)}

    return run, unpack


def bench(iters=8, repeat=1, **inputs):
    """Time repeated on-device executions; returns (best_ns, core0 outputs)."""
    run, unpack = _make_runner(repeat=repeat, **inputs)
    best, outs = float("inf"), None
    for _ in range(iters):
        ns, outs = run()
        best = min(best, ns)
    return best, unpack(outs)
